# revision 26
# baseline (speedup 1.0000x reference)
"""Trainium2 Bass kernel for a single-layer transformer encoder
(pos-embed + causal/pad-masked MHA + 2x layernorm + relu FFN).

Contract: kernel(**inputs) takes the FULL unsharded inputs (as produced
by the problem's setup_inputs) and returns the FULL [16, 1024, 512] f32
output. Internally: data-parallel over the batch dim across 8
NeuronCores (2 batches per core), single SPMD NEFF.

Design notes:
 - All matmuls run in bf16 with f32 PSUM accumulation.
 - Scores are computed transposed (S^T[k, q]) so the softmax k-reduction
   can ride the TensorEngine and attn@V needs no transposes. The
   denominator is tree-summed on DVE (bf16) and finished with a single
   ones-matmul per (head, q-window).
 - Blocks strictly above the causal diagonal are skipped entirely; the
   reference's pad-row behaviour (fully-masked rows softmax to uniform
   1/L attention) is reproduced by overwriting padded query columns of
   ctx^T with mean_k(V)/L via copy_predicated. mean(V) is derived as
   (sum_tok x) @ W_v on the fly.
 - x = seq + pos_table is precomputed on the host (layout prep) and
   shipped in both natural (f32) and transposed (bf16) layouts.
"""

import sys

for _p in ("/opt/trn_rl_repo",):
    if _p not in sys.path:
        sys.path.insert(0, _p)

import numpy as np
import ml_dtypes

import concourse.bass as bass
import concourse.tile as tile
from concourse import bacc, mybir
from concourse.bass_utils import run_bass_kernel_spmd

BF16 = ml_dtypes.bfloat16

N_CORES = 8
B = 16
L = 1024
D = 512
H = 8
DK = 128
BPC = B // N_CORES  # batches per core
LN_EPS = 1e-5
INV_TEMP = 1.0 / (np.sqrt(128.0) + 1e-6)

F32 = mybir.dt.float32
BF = mybir.dt.bfloat16

_cache = {}

import os
MM_BUFS = int(os.environ.get("K_MM_BUFS", "4"))
ACC_BUFS = int(os.environ.get("K_ACC_BUFS", "2"))
DN_BUFS = int(os.environ.get("K_DN_BUFS", "1"))
SMALL_BUFS = int(os.environ.get("K_SMALL_BUFS", "1"))
QT_BUFS = int(os.environ.get("K_QT_BUFS", "1"))
KT_BUFS = int(os.environ.get("K_KT_BUFS", "2"))
V_BUFS = int(os.environ.get("K_V_BUFS", "1"))
QW = int(os.environ.get("K_QW", "512"))
DEEP_DN = int(os.environ.get("K_DEEP_DN", "0"))
EXPS_BUFS = int(os.environ.get("K_EXPS_BUFS", "10"))


def _build(affine, use_b2, reps=1):
    """Build + compile the SPMD program. Returns nc."""
    nc = bacc.Bacc("TRN2", target_bir_lowering=False, debug=False,
                   num_devices=N_CORES)

    # ---- DRAM I/O ----
    xnat = nc.dram_tensor("xnat", [BPC, 128, 8, 512], BF, kind="ExternalInput")
    xtr = nc.dram_tensor("xtr", [BPC, 128, 4, 1024], BF, kind="ExternalInput")
    padb = nc.dram_tensor("padb", [BPC, 1, L], mybir.dt.uint8, kind="ExternalInput")
    wq = nc.dram_tensor("wq", [D, H * DK], BF, kind="ExternalInput")
    wk = nc.dram_tensor("wk", [D, H * DK], BF, kind="ExternalInput")
    wv = nc.dram_tensor("wv", [D, H * DK], BF, kind="ExternalInput")
    wo = nc.dram_tensor("wo", [H * DK, D], BF, kind="ExternalInput")
    w1t = nc.dram_tensor("w1t", [D, D], BF, kind="ExternalInput")
    w2t = nc.dram_tensor("w2t", [D, D], BF, kind="ExternalInput")
    b1c = nc.dram_tensor("b1c", [D, 1], F32, kind="ExternalInput")
    b2r = nc.dram_tensor("b2r", [1, D], F32, kind="ExternalInput")
    lng = nc.dram_tensor("lng", [1, D], F32, kind="ExternalInput")
    lnb = nc.dram_tensor("lnb", [1, D], F32, kind="ExternalInput")
    out = nc.dram_tensor("out", [BPC, L, D], F32, kind="ExternalOutput")

    # ---- inline constants ----
    kk = np.arange(128)[:, None]
    qq = np.arange(128)[None, :]
    tri_np = (kk <= qq).astype(BF16)  # [128,128] causal block mask
    tri_d = nc.inline_tensor(tri_np, name="tri")
    ones_d = nc.inline_tensor(np.ones((128, 128), dtype=BF16), name="onesc")
    ident_d = nc.inline_tensor(np.eye(128, dtype=BF16), name="ident")

    def bcast_dram(ap2d, p=128):
        # [1, N] dram AP -> [p, N] partition-broadcast AP for DMA
        return bass.AP(tensor=ap2d.tensor, offset=ap2d.offset,
                       ap=[[0, p]] + list(ap2d.ap[1:]))

    Exp = mybir.ActivationFunctionType.Exp
    Sqrt = mybir.ActivationFunctionType.Sqrt
    mult = mybir.AluOpType.mult
    addop = mybir.AluOpType.add
    maxop = mybir.AluOpType.max
    AxF = mybir.AxisListType.X

    with tile.TileContext(nc) as tc:
      with (
        tc.tile_pool(name="const", bufs=1) as cpool,
        tc.tile_pool(name="big", bufs=1) as bpool,
        tc.tile_pool(name="work", bufs=2) as wpool,
        tc.tile_pool(name="psum", bufs=2, space="PSUM") as pp,
      ):
        # ---- weights / constants ----
        wq_s = cpool.tile([128, 4, 1024], BF, name="wq_s")
        wk_s = cpool.tile([128, 4, 1024], BF, name="wk_s")
        wv_s = cpool.tile([128, 4, 1024], BF, name="wv_s")
        wo_s = cpool.tile([128, 8, 512], BF, name="wo_s")
        w1t_s = cpool.tile([128, 4, 512], BF, name="w1t_s")
        w2t_s = cpool.tile([128, 4, 512], BF, name="w2t_s")
        tri_s = cpool.tile([128, 128], BF, name="tri_s")
        ones_s = cpool.tile([128, 128], BF, name="ones_s")
        ident_s = cpool.tile([128, 128], BF, name="ident_s")
        b1c_s = cpool.tile([128, 4], F32, name="b1c_s")
        padb_s = cpool.tile([128, BPC, 1024], mybir.dt.uint8, name="padb_s")
        eps_s = cpool.tile([128, 1], F32, name="eps_s")
        nc.vector.memset(eps_s, LN_EPS)

        xT0 = bpool.tile([128, 4, 1024], BF, name="xT0pre", tag="xT")
        nc.sync.dma_start(out=xT0, in_=xtr.ap()[0])
        for dc in range(4):
            nc.sync.dma_start(out=wq_s[:, dc, :],
                              in_=wq.ap().rearrange("(c p) n -> p c n", p=128)[:, dc, :])
            nc.scalar.dma_start(out=wk_s[:, dc, :],
                                in_=wk.ap().rearrange("(c p) n -> p c n", p=128)[:, dc, :])
        nc.scalar.dma_start(out=wv_s, in_=wv.ap().rearrange("(c p) n -> p c n", p=128))
        nc.scalar.dma_start(out=ones_s, in_=ones_d.ap())
        nc.scalar.dma_start(out=tri_s, in_=tri_d.ap())
        nc.scalar.dma_start(out=wo_s, in_=wo.ap().rearrange("(c p) n -> p c n", p=128))
        nc.scalar.dma_start(out=w1t_s, in_=w1t.ap().rearrange("(c p) n -> p c n", p=128))
        nc.scalar.dma_start(out=w2t_s, in_=w2t.ap().rearrange("(c p) n -> p c n", p=128))
        nc.scalar.dma_start(out=ident_s, in_=ident_d.ap())
        nc.scalar.dma_start(out=b1c_s, in_=b1c.ap().rearrange("(c p) one -> p (c one)", p=128))
        for b in range(BPC):
            nc.gpsimd.dma_start(out=padb_s[:, b, :], in_=bcast_dram(padb.ap()[b]))
        if use_b2:
            b2_s = cpool.tile([128, 512], F32, name="b2_s")
            nc.gpsimd.dma_start(out=b2_s, in_=bcast_dram(b2r.ap()))
        if affine:
            g_s = cpool.tile([128, 512], F32, name="g_s")
            bb_s = cpool.tile([128, 512], F32, name="bb_s")
            nc.gpsimd.dma_start(out=g_s, in_=bcast_dram(lng.ap()))
            nc.gpsimd.dma_start(out=bb_s, in_=bcast_dram(lnb.ap()))

        def layer_norm(dst, src, small, dve_apply=False):
            # dst = (src - mean) * rstd [* g + b]; engine hops limited to
            # DVE -> ACT(sqrt, tiny) -> DVE.
            stats = small.tile([128, 6], F32, tag="lnstats", bufs=4)
            mv = small.tile([128, 2], F32, tag="lnmv", bufs=4)
            sd = small.tile([128, 1], F32, tag="lnsd", bufs=4)
            rs = small.tile([128, 1], F32, tag="lnrs", bufs=4)
            nmr = small.tile([128, 1], F32, tag="lnnmr", bufs=4)
            nc.vector.bn_stats(out=stats, in_=src)
            nc.vector.bn_aggr(out=mv, in_=stats)
            nc.scalar.activation(out=sd, in_=mv[:, 1:2], func=Sqrt, bias=eps_s)
            nc.vector.reciprocal(out=rs, in_=sd)
            nc.vector.scalar_tensor_tensor(out=nmr, in0=mv[:, 0:1], scalar=-1.0,
                                           in1=rs, op0=mult, op1=mult)
            if dve_apply:
                nc.vector.tensor_scalar(out=dst, in0=src, scalar1=rs,
                                        scalar2=nmr, op0=mult, op1=addop)
            else:
                nc.scalar.activation(out=dst, in_=src,
                                     func=mybir.ActivationFunctionType.Identity,
                                     bias=nmr, scale=rs)
            if affine:
                nc.vector.tensor_mul(dst, dst, g_s)
                nc.vector.tensor_add(dst, dst, bb_s)

        # ---- per-batch processing ----
        for _rep in range(reps):
          for b in range(BPC):
            # stage 1: load x (natural f32) and x^T (bf16); pos added on host
            if b == 0 and _rep == 0:
                xT = xT0
            else:
                xT = bpool.tile([128, 4, 1024], BF, name=f"xT{b}", tag="xT")
                nc.sync.dma_start(out=xT, in_=xtr.ap()[b])
            # residual x (bf16, partition-major), one DMA per batch
            xn_all = bpool.tile([128, 8, 512], BF, name=f"xn{b}", tag="XN",
                                bufs=2)
            nc.scalar.dma_start(out=xn_all, in_=xnat.ap()[b])

            # stage 2: projections
            qt_sb = bpool.tile([128, 8, 1024], BF, name=f"qt{b}", tag="QT", bufs=QT_BUFS)
            kt_sb = bpool.tile([128, 8, 1024], BF, name=f"kt{b}", tag="KT", bufs=KT_BUFS)
            v_sb = bpool.tile([128, 8, 1024], BF, name=f"v{b}", tag="V", bufs=V_BUFS)
            for w_s, dst in ((wq_s, qt_sb), (wk_s, kt_sb)):
                for hc in range(8):
                    ps0 = pp.tile([128, 512], F32, tag="mm", bufs=MM_BUFS)
                    ps1 = pp.tile([128, 512], F32, tag="mm", bufs=MM_BUFS)
                    for dc in range(4):
                        nc.tensor.matmul(ps0, lhsT=w_s[:, dc, hc * 128:(hc + 1) * 128],
                                         rhs=xT[:, dc, 0:512],
                                         start=(dc == 0), stop=(dc == 3))
                        nc.tensor.matmul(ps1, lhsT=w_s[:, dc, hc * 128:(hc + 1) * 128],
                                         rhs=xT[:, dc, 512:1024],
                                         start=(dc == 0), stop=(dc == 3))
                    nc.vector.tensor_copy(dst[:, hc, 0:512], ps0)
                    nc.vector.tensor_copy(dst[:, hc, 512:1024], ps1)
            for tt in range(8):  # V natural: token chunk tt
                ps0 = pp.tile([128, 512], F32, tag="mm", bufs=MM_BUFS)
                ps1 = pp.tile([128, 512], F32, tag="mm", bufs=MM_BUFS)
                for dc in range(4):
                    nc.tensor.matmul(ps0, lhsT=xT[:, dc, tt * 128:(tt + 1) * 128],
                                     rhs=wv_s[:, dc, 0:512],
                                     start=(dc == 0), stop=(dc == 3))
                    nc.tensor.matmul(ps1, lhsT=xT[:, dc, tt * 128:(tt + 1) * 128],
                                     rhs=wv_s[:, dc, 512:1024],
                                     start=(dc == 0), stop=(dc == 3))
                nc.vector.tensor_copy(v_sb[:, tt, 0:512], ps0)
                nc.vector.tensor_copy(v_sb[:, tt, 512:1024], ps1)

            # mean(V)/L columns for the pad-row fixup:
            # meanVT[hd, h] = (1/L) * sum_d Wv[d, (h,hd)] * xsum[d]
            xsum = bpool.tile([128, 4], F32, name=f"xsum{b}", tag="xsum")
            for dc in range(4):
                nc.vector.reduce_sum(out=xsum[:, dc:dc + 1], in_=xT[:, dc, :],
                                     axis=AxF)
            xsum_bf = bpool.tile([128, 4], BF, name=f"xsumb{b}", tag="xsumb")
            nc.vector.tensor_copy(xsum_bf, xsum)
            meanv = bpool.tile([128, 8], BF, name=f"meanv{b}", tag="meanv")
            mv_ps = pp.tile([128, 8], F32, tag="small", bufs=SMALL_BUFS)
            for h in range(8):
                for dc in range(4):
                    nc.tensor.matmul(mv_ps[:, h:h + 1],
                                     lhsT=wv_s[:, dc, h * 128:(h + 1) * 128],
                                     rhs=xsum_bf[:, dc:dc + 1],
                                     start=(dc == 0), stop=(dc == 3))
            nc.scalar.mul(meanv, mv_ps, 1.0 / L)

            # stage 3: attention (scores transposed S^T[k, q])
            ctx_sb = bpool.tile([128, 8, 1024], BF, name=f"ctx{b}", tag="CTX")
            for h in range(8):
                hs = slice(h * 128, (h + 1) * 128)
                cx0 = pp.tile([128, 512], F32, tag="acc", bufs=ACC_BUFS)
                cx1 = pp.tile([128, 512], F32, tag="acc", bufs=ACC_BUFS)
                ex0s, ex1s = [], []
                for i in range(8):
                    ks = slice(i * 128, (i + 1) * 128)
                    lo = 128 * i
                    if i < 4:
                        sp0 = pp.tile([128, 512], F32, tag="mm", bufs=MM_BUFS)
                        sp1 = pp.tile([128, 512], F32, tag="mm", bufs=MM_BUFS)
                        nc.tensor.matmul(sp0[:, lo:512],
                                         lhsT=kt_sb[:, h, ks],
                                         rhs=qt_sb[:, h, lo:512],
                                         start=True, stop=True)
                        nc.tensor.matmul(sp1, lhsT=kt_sb[:, h, ks],
                                         rhs=qt_sb[:, h, 512:1024],
                                         start=True, stop=True)
                        ex0 = wpool.tile([128, 512], BF, tag="expS",
                                         bufs=EXPS_BUFS)
                        ex1 = wpool.tile([128, 512], BF, tag="expS",
                                         bufs=EXPS_BUFS)
                        nc.scalar.activation(out=ex0[:, lo:512],
                                             in_=sp0[:, lo:512],
                                             func=Exp, scale=INV_TEMP)
                        nc.scalar.activation(out=ex1, in_=sp1,
                                             func=Exp, scale=INV_TEMP)
                        nc.vector.tensor_mul(ex0[:, lo:lo + 128],
                                             ex0[:, lo:lo + 128], tri_s)
                        nc.tensor.matmul(cx0[:, lo:512], lhsT=v_sb[:, i, hs],
                                         rhs=ex0[:, lo:512],
                                         start=(i == 0), stop=(i == 3),
                                         skip_group_check=True)
                        nc.tensor.matmul(cx1, lhsT=v_sb[:, i, hs], rhs=ex1,
                                         start=(i == 0), stop=(i == 7),
                                         skip_group_check=True)
                        ex0s.append(ex0)
                        ex1s.append(ex1)
                    else:
                        wlo = lo - 512
                        sp1 = pp.tile([128, 512], F32, tag="mm", bufs=MM_BUFS)
                        nc.tensor.matmul(sp1[:, wlo:512],
                                         lhsT=kt_sb[:, h, ks],
                                         rhs=qt_sb[:, h, lo:1024],
                                         start=True, stop=True)
                        ex1 = wpool.tile([128, 512], BF, tag="expS",
                                         bufs=EXPS_BUFS)
                        nc.scalar.activation(out=ex1[:, wlo:512],
                                             in_=sp1[:, wlo:512],
                                             func=Exp, scale=INV_TEMP)
                        nc.vector.tensor_mul(ex1[:, wlo:wlo + 128],
                                             ex1[:, wlo:wlo + 128], tri_s)
                        nc.tensor.matmul(cx1[:, wlo:512],
                                         lhsT=v_sb[:, i, hs],
                                         rhs=ex1[:, wlo:512],
                                         start=False, stop=(i == 7),
                                         skip_group_check=True)
                        ex1s.append(ex1)

                # window-0 tree: serial partial sums into ex0s[0]
                e0 = ex0s[0]
                nc.gpsimd.tensor_add(e0[:, 128:512], e0[:, 128:512],
                                     ex0s[1][:, 128:512])
                nc.gpsimd.tensor_add(e0[:, 256:512], e0[:, 256:512],
                                     ex0s[2][:, 256:512])
                nc.vector.tensor_add(e0[:, 384:512], e0[:, 384:512],
                                     ex0s[3][:, 384:512])
                dn0 = pp.tile([128, 512], F32, tag="dn", bufs=DN_BUFS)
                nc.tensor.matmul(dn0, lhsT=ones_s, rhs=e0,
                                 start=True, stop=True)
                nc.any.tensor_copy(ctx_sb[:, h, 0:512], cx0)
                rcp0 = wpool.tile([128, 512], F32, tag="rcp", bufs=2)
                nc.vector.reciprocal(out=rcp0, in_=dn0)
                nc.vector.tensor_mul(ctx_sb[:, h, 0:512],
                                     ctx_sb[:, h, 0:512], rcp0)
                # window-1 tree
                f0, f5 = ex1s[0], ex1s[5]
                nc.gpsimd.tensor_add(f0, f0, ex1s[1])
                nc.gpsimd.tensor_add(ex1s[2], ex1s[2], ex1s[3])
                nc.gpsimd.tensor_add(f5[:, 256:512], f5[:, 256:512],
                                     ex1s[6][:, 256:512])
                nc.gpsimd.tensor_add(f5[:, 384:512], f5[:, 384:512],
                                     ex1s[7][:, 384:512])
                nc.vector.tensor_add(f0, f0, ex1s[2])
                nc.vector.tensor_add(f0, f0, ex1s[4])
                nc.vector.tensor_add(f0[:, 128:512], f0[:, 128:512],
                                     f5[:, 128:512])
                dn1 = pp.tile([128, 512], F32, tag="dn", bufs=DN_BUFS)
                nc.tensor.matmul(dn1, lhsT=ones_s, rhs=f0,
                                 start=True, stop=True)
                nc.any.tensor_copy(ctx_sb[:, h, 512:1024], cx1)
                rcp1 = wpool.tile([128, 512], F32, tag="rcp", bufs=2)
                nc.vector.reciprocal(out=rcp1, in_=dn1)
                nc.vector.tensor_mul(ctx_sb[:, h, 512:1024],
                                     ctx_sb[:, h, 512:1024], rcp1)
                nc.vector.copy_predicated(
                    out=ctx_sb[:, h, :],
                    mask=padb_s[:, b, :],
                    data=meanv[:, h:h + 1].to_broadcast([128, 1024]))

            # stage 4: W_o, residual, LN1 (bf16 X), fused X^T transposes
            Xn = bpool.tile([128, 8, 512], BF, name=f"Xn{b}", tag="Xn")
            xt_sb = bpool.tile([128, 4, 1024], BF, name=f"xt{b}", tag="XT")
            for qt in range(8):
                qs = slice(qt * 128, (qt + 1) * 128)
                va_ps = pp.tile([128, 512], F32, tag="mm", bufs=MM_BUFS)
                for h in range(8):
                    nc.tensor.matmul(va_ps, lhsT=ctx_sb[:, h, qs],
                                     rhs=wo_s[:, h, :],
                                     start=(h == 0), stop=(h == 7))
                r1 = wpool.tile([128, 512], F32, tag="r1", bufs=2)
                nc.vector.tensor_add(r1, va_ps, xn_all[:, qt, :])
                layer_norm(Xn[:, qt, :], r1, wpool, dve_apply=True)
                tp_ps = pp.tile([128, 4, 128], BF, tag="small", bufs=SMALL_BUFS)
                for dc in range(4):
                    nc.tensor.transpose(tp_ps[:, dc, :],
                                        Xn[:, qt, dc * 128:(dc + 1) * 128],
                                        ident_s)
                nc.vector.tensor_copy(xt_sb[:, :, qt * 128:(qt + 1) * 128], tp_ps)

            # stage 6: FFN1 (relu^T layout [f, q]); bias+relu on DVE
            rel_sb = bpool.tile([128, 4, 1024], BF, name=f"rel{b}", tag="REL")
            for fc in range(4):
                f_ps0 = pp.tile([128, 512], F32, tag="mm", bufs=MM_BUFS)
                f_ps1 = pp.tile([128, 512], F32, tag="mm", bufs=MM_BUFS)
                for dc in range(4):
                    nc.tensor.matmul(f_ps0,
                                     lhsT=w1t_s[:, dc, fc * 128:(fc + 1) * 128],
                                     rhs=xt_sb[:, dc, 0:512],
                                     start=(dc == 0), stop=(dc == 3))
                    nc.tensor.matmul(f_ps1,
                                     lhsT=w1t_s[:, dc, fc * 128:(fc + 1) * 128],
                                     rhs=xt_sb[:, dc, 512:1024],
                                     start=(dc == 0), stop=(dc == 3))
                nc.vector.tensor_scalar(out=rel_sb[:, fc, 0:512], in0=f_ps0,
                                        scalar1=b1c_s[:, fc:fc + 1],
                                        scalar2=0.0, op0=addop, op1=maxop)
                nc.vector.tensor_scalar(out=rel_sb[:, fc, 512:1024],
                                        in0=f_ps1,
                                        scalar1=b1c_s[:, fc:fc + 1],
                                        scalar2=0.0, op0=addop, op1=maxop)

            # stage 7: FFN2, residual, LN2, store
            for qt in range(8):
                qs = slice(qt * 128, (qt + 1) * 128)
                ff_ps = pp.tile([128, 512], F32, tag="mm", bufs=MM_BUFS)
                for fc in range(4):
                    nc.tensor.matmul(ff_ps, lhsT=rel_sb[:, fc, qs],
                                     rhs=w2t_s[:, fc, :],
                                     start=(fc == 0), stop=(fc == 3))
                r2 = wpool.tile([128, 512], F32, tag="r2", bufs=2)
                nc.vector.tensor_add(r2, ff_ps, Xn[:, qt, :])
                if use_b2:
                    nc.vector.tensor_add(r2, r2, b2_s)
                o_t = wpool.tile([128, 512], F32, tag="out", bufs=2)
                layer_norm(o_t, r2, wpool, dve_apply=True)
                nc.sync.dma_start(out=out.ap()[b, qt * 128:(qt + 1) * 128, :],
                                  in_=o_t)

    nc.compile()
    return nc


def _get_nc(affine, use_b2, reps=1):
    key = (affine, use_b2, reps)
    if key not in _cache:
        _cache[key] = _build(affine, use_b2, reps)
    return _cache[key]


def _prep_inputs(seq_h, pad_mask, pos_table, W_q, W_k, W_v, W_o, w1, b1, w2,
                 b2, ln_g, ln_b):
    seq_h = np.asarray(seq_h, dtype=np.float32)
    pad_mask = np.asarray(pad_mask)
    affine = not (np.all(np.asarray(ln_g) == 1.0) and np.all(np.asarray(ln_b) == 0.0))
    use_b2 = bool(np.any(np.asarray(b2) != 0.0))

    common = {
        "wq": np.asarray(W_q, np.float32).astype(BF16),
        "wk": np.asarray(W_k, np.float32).astype(BF16),
        "wv": np.asarray(W_v, np.float32).astype(BF16),
        "wo": np.asarray(W_o, np.float32).astype(BF16),
        "w1t": np.ascontiguousarray(np.asarray(w1, np.float32).T).astype(BF16),
        "w2t": np.ascontiguousarray(np.asarray(w2, np.float32).T).astype(BF16),
        "b1c": np.asarray(b1, np.float32).reshape(D, 1),
        "b2r": np.asarray(b2, np.float32).reshape(1, D),
        "lng": np.asarray(ln_g, np.float32).reshape(1, D),
        "lnb": np.asarray(ln_b, np.float32).reshape(1, D),
    }
    x = seq_h + np.asarray(pos_table, np.float32)[:L][None]
    # natural x, partition-major: [B, 128(p), 8(qt), 512(d)], bf16
    xnat_pm = np.ascontiguousarray(
        x.reshape(B, 8, 128, D).transpose(0, 2, 1, 3)).astype(BF16)
    # x^T, partition-major: [B, 128(p), 4(dc), 1024(tok)], bf16
    xT = np.ascontiguousarray(x.transpose(0, 2, 1)).astype(BF16)
    xtr_pm = np.ascontiguousarray(
        xT.reshape(B, 4, 128, L).transpose(0, 2, 1, 3))
    padb = pad_mask.astype(np.uint8).reshape(B, 1, L)

    in_maps = []
    for c in range(N_CORES):
        sl = slice(c * BPC, (c + 1) * BPC)
        m = dict(common)
        m["xnat"] = np.ascontiguousarray(xnat_pm[sl])
        m["xtr"] = np.ascontiguousarray(xtr_pm[sl])
        m["padb"] = np.ascontiguousarray(padb[sl])
        in_maps.append(m)
    return in_maps, affine, use_b2


def _run_once(nc, in_maps):
    res = run_bass_kernel_spmd(nc, in_maps, core_ids=list(range(N_CORES)))
    return np.concatenate([np.asarray(r["out"]) for r in res.results], axis=0)


def kernel(**inputs) -> np.ndarray:
    in_maps, affine, use_b2 = _prep_inputs(**inputs)
    nc = _get_nc(affine, use_b2)
    # Device execution is deterministic; a rare infra flake can return
    # stale data for some cores. Run twice and majority-vote on mismatch.
    a = _run_once(nc, in_maps)
    b = _run_once(nc, in_maps)
    if np.array_equal(a, b):
        return a
    c = _run_once(nc, in_maps)
    if np.array_equal(a, c):
        return a
    return c if np.array_equal(b, c) else b

